# revision 20
# baseline (speedup 1.0000x reference)
"""Trainium2 Bass kernel for nn_L2MLoRAqkv (MoE-routed LoRA QKV projection).

Math (per batch b, expert i = idx[b,0]):
    qkv = x @ W.T + bias
    qkv[:, :D]  += (x @ A_q[i]) @ B_q[i] * SCALE
    qkv[:, -D:] += (x @ A_v[i]) @ B_v[i] * SCALE

Strategy: data-parallel over the batch dim (1 batch per NeuronCore, 8 cores).
On the host we gather each batch's expert and fold the rank-8 LoRA update
into the (transposed) projection weight in float64:
    W_eff[b] = W.T; W_eff[:, :D] += A_q[i] @ B_q[i]; W_eff[:, -D:] += A_v[i] @ B_v[i]
so the device kernel is a single dense GEMM per core:
    Y[4096, 3072] = X[4096, 1024] @ W_eff[1024, 3072] + bias

All GEMM operands move as bf16 (PSUM accumulation stays f32): halves HBM
traffic vs f32 and keeps the PE at 1 cycle/row.  X and W_eff are fully
SBUF-resident.  Loads ride the sync HWDGE ring, ordered so the first
accumulation group's tiles land first (w[n0] and x[c0] interleaved
k-by-k); bias + output stores ride the scalar ring so loads are never
queued behind store semaphores.  The first chunk's compute is k-outer
across 4 PSUM banks so matmuls chase the DMA arrival order.
"""

import os
import sys

import numpy as np

for _p in ("/opt/trn_rl_repo",):
    if _p not in sys.path and os.path.isdir(_p):
        sys.path.insert(0, _p)

B = 8          # batches == cores
T = 4096       # tokens per batch
D = 1024       # model dim (contraction K)
N3 = 3072      # qkv output dim
P = 128        # SBUF partitions
NT = 512       # n-tile (one fp32 PSUM bank)
CH = 512       # token chunk (DMA granule for x)
KT = D // P        # 8 k-tiles
NN = N3 // NT      # 6 n-tiles
TT = CH // P       # 4 token sub-tiles per chunk
NCH = T // CH      # 8 chunks
SCALE = 8.0 / 8.0

_NC_CACHE = {}


def _build():
    import concourse.tile as tile
    from concourse import bacc, mybir

    bf16 = mybir.dt.bfloat16
    f32 = mybir.dt.float32

    nc = bacc.Bacc(
        "TRN2",
        target_bir_lowering=False,
        debug=False,
        enable_asserts=False,
        num_devices=B,
    )
    xt = nc.dram_tensor("xt", [D, T], bf16, kind="ExternalInput").ap()
    weff = nc.dram_tensor("weff", [D, N3], bf16, kind="ExternalInput").ap()
    biasr = nc.dram_tensor("biasr", [1, N3], bf16, kind="ExternalInput").ap()
    y = nc.dram_tensor("y", [T, N3], bf16, kind="ExternalOutput").ap()

    with tile.TileContext(nc) as tc:
        with tc.tile_pool(name="const", bufs=1) as const_pool, \
             tc.tile_pool(name="outp", bufs=6) as out_pool, \
             tc.tile_pool(name="ps", bufs=6, space="PSUM") as psum_pool, \
             tc.tile_pool(name="psb", bufs=2, space="PSUM") as psum_b_pool:

            # k-slice k of x lives at cols [k*T, (k+1)*T); of w at [k*N3, ...).
            x_sb = const_pool.tile([P, KT * T], bf16)
            w_sb = const_pool.tile([P, KT * N3], bf16)
            bias_sb = const_pool.tile([P, N3], f32)
            bias_row = const_pool.tile([1, N3], bf16)
            ones_sb = const_pool.tile([1, P], bf16)

            def load_x_c(c, eng):
                for k in range(KT):
                    eng.dma_start(
                        x_sb[:, k * T + c * CH : k * T + (c + 1) * CH],
                        xt[k * P : (k + 1) * P, c * CH : (c + 1) * CH],
                    )

            # Startup-critical stream.  Compute consumes (w[k], x[k, c0])
            # k-major, so w ships as whole k-rows (6KB partition lines move
            # at ~330GB/s vs ~200GB/s for 1KB-line pieces) split across both
            # rings; x chunk 0 rides the scalar ring as small pieces for
            # progressive availability.  Everything else queues behind the
            # critical loads — HWDGE FIFO order is the priority mechanism.
            nc.scalar.dma_start(bias_row[:], biasr[:])
            load_x_c(0, nc.scalar)
            for k in range(KT):
                eng = nc.scalar if k % 2 else nc.sync
                eng.dma_start(
                    w_sb[:, k * N3 : (k + 1) * N3],
                    weff[k * P : (k + 1) * P, :],
                )
            for k in range(KT):
                eng = nc.scalar if k % 2 else nc.sync
                eng.dma_start(
                    x_sb[:, k * T + CH : (k + 1) * T],
                    xt[k * P : (k + 1) * P, CH:T],
                )

            # One [128, 3072] output tile per token tile: the six bias-add
            # drains fill it slice by slice, then two half-row stores (3KB
            # partition lines) ship it on both rings in parallel.
            obs = {}

            def drain(ps, tg, n):
                if tg not in obs:
                    obs[tg] = out_pool.tile([P, N3], bf16, tag="ob", name="ob")
                ob = obs[tg]
                nc.vector.tensor_add(
                    ob[:, n * NT : (n + 1) * NT],
                    ps[:],
                    bias_sb[:, n * NT : (n + 1) * NT],
                )

            def store(tg, final=False):
                ob = obs.pop(tg)
                if final:
                    # Six slice-stores alternating rings: the first slices
                    # ship while the last drains still run, shrinking the
                    # kernel tail.
                    for n in range(NN):
                        eng = nc.sync if n % 2 else nc.scalar
                        eng.dma_start(
                            y[tg * P : (tg + 1) * P, n * NT : (n + 1) * NT],
                            ob[:, n * NT : (n + 1) * NT],
                        )
                    return
                half = N3 // 2
                nc.scalar.dma_start(
                    y[tg * P : (tg + 1) * P, 0:half], ob[:, 0:half]
                )
                nc.sync.dma_start(
                    y[tg * P : (tg + 1) * P, half:N3], ob[:, half:N3]
                )

            def mm(ps, tg, n, k):
                nc.tensor.matmul(
                    ps[:],
                    lhsT=x_sb[:, k * T + tg * P : k * T + (tg + 1) * P],
                    rhs=w_sb[:, k * N3 + n * NT : k * N3 + (n + 1) * NT],
                    start=(k == 0),
                    stop=(k == KT - 1),
                )

            # ~30 dependency-free warm-up matmuls on the ones tile keep the
            # PE busy from engine-up (~7us) until the first data lands
            # (~10us), so the HAM clock gate reaches 8/8 by the time real
            # work is streaming.  Output goes to a PSUM bank that is never
            # read.
            nc.vector.memset(ones_sb[:], 1.0)
            wub = psum_b_pool.tile([P, P], f32, tag="psb", name="psb")
            for _ in range(30):
                nc.tensor.matmul(
                    wub[:], lhsT=ones_sb[:], rhs=ones_sb[:],
                    start=True, stop=True,
                )

            # Replicate bias across partitions on-chip: ones[1,128].T @
            # bias_row[1,512] fills [128,512].  12KB of HBM instead of 1.5MB,
            # and the PE does it in ~1.3us while the first block streams.
            def bias_bcast():
                for n in range(NN):
                    psb = psum_b_pool.tile([P, NT], f32, tag="psb", name="psb")
                    nc.tensor.matmul(
                        psb[:],
                        lhsT=ones_sb[:],
                        rhs=bias_row[:, n * NT : (n + 1) * NT],
                        start=True,
                        stop=True,
                    )
                    nc.vector.tensor_copy(
                        bias_sb[:, n * NT : (n + 1) * NT], psb[:]
                    )

            # Everything else: k-outer / n-inner with one PSUM bank per n, so
            # the stationary operand (lhsT = x tile) is identical across the
            # n-consecutive matmuls — the PE's weight path stays quiet.
            def t_block(tg, first=False, final=False):
                pss = [psum_pool.tile([P, NT], f32, tag="ps", name="ps")
                       for _ in range(NN)]
                for k in range(KT):
                    for n in range(NN):
                        mm(pss[n], tg, n, k)
                    if first and k == 0:
                        bias_bcast()
                for n in range(NN):
                    drain(pss[n], tg, n)
                store(tg, final=final)

            for tg in range(T // P):
                t_block(tg, first=(tg == 0), final=(tg == T // P - 1))
    nc.compile()
    return nc


def _get_nc():
    if "v2" not in _NC_CACHE:
        _NC_CACHE["v2"] = _build()
    return _NC_CACHE["v2"]


def _prep_in_maps(inputs):
    import ml_dtypes

    bf16 = ml_dtypes.bfloat16

    x = np.asarray(inputs["x"], dtype=np.float32)
    weight = np.asarray(inputs["weight"], dtype=np.float32)
    bias = np.asarray(inputs["bias"], dtype=np.float32)
    aq = np.asarray(inputs["A_q_pool"], dtype=np.float32)
    bq = np.asarray(inputs["B_q_pool"], dtype=np.float32)
    av = np.asarray(inputs["A_v_pool"], dtype=np.float32)
    bv = np.asarray(inputs["B_v_pool"], dtype=np.float32)
    idx = np.asarray(inputs["idx"]).reshape(B, -1)[:, 0].astype(np.int64)

    wt64 = weight.T.astype(np.float64)  # [D, N3]
    biasr = np.ascontiguousarray(bias.reshape(1, N3))
    xts = x.transpose(0, 2, 1)  # [B, D, T] strided view

    in_maps = []
    for b in range(B):
        i = int(idx[b])
        weff = wt64.copy()
        weff[:, :D] += SCALE * (aq[i].astype(np.float64) @ bq[i].astype(np.float64))
        weff[:, N3 - D:] += SCALE * (av[i].astype(np.float64) @ bv[i].astype(np.float64))
        in_maps.append({
            "xt": np.ascontiguousarray(xts[b]).astype(bf16),
            "weff": weff.astype(np.float32).astype(bf16),
            "biasr": biasr.astype(bf16),
        })
    return in_maps


def _run(in_maps, trace=False, **kwargs):
    from concourse.bass_utils import run_bass_kernel_spmd

    nc = _get_nc()
    return run_bass_kernel_spmd(
        nc, in_maps, core_ids=list(range(B)), trace=trace, **kwargs
    )


def kernel(**inputs):
    res = _run(_prep_in_maps(inputs), trace=False)
    return np.stack(
        [np.asarray(r["y"], dtype=np.float32) for r in res.results], axis=0
    )


# revision 21
# speedup vs baseline: 1.0439x; 1.0439x over previous
"""Trainium2 Bass kernel for nn_L2MLoRAqkv (MoE-routed LoRA QKV projection).

Math (per batch b, expert i = idx[b,0]):
    qkv = x @ W.T + bias
    qkv[:, :D]  += (x @ A_q[i]) @ B_q[i] * SCALE
    qkv[:, -D:] += (x @ A_v[i]) @ B_v[i] * SCALE

Strategy: data-parallel over the batch dim (1 batch per NeuronCore, 8 cores).
On the host we gather each batch's expert and fold the rank-8 LoRA update
into the (transposed) projection weight in float64:
    W_eff[b] = W.T; W_eff[:, :D] += A_q[i] @ B_q[i]; W_eff[:, -D:] += A_v[i] @ B_v[i]
so the device kernel is a single dense GEMM per core:
    Y[4096, 3072] = X[4096, 1024] @ W_eff[1024, 3072] + bias

All GEMM operands move as bf16 (PSUM accumulation stays f32): halves HBM
traffic vs f32 and keeps the PE at 1 cycle/row.  X and W_eff are fully
SBUF-resident.  Loads ride the sync HWDGE ring, ordered so the first
accumulation group's tiles land first (w[n0] and x[c0] interleaved
k-by-k); bias + output stores ride the scalar ring so loads are never
queued behind store semaphores.  The first chunk's compute is k-outer
across 4 PSUM banks so matmuls chase the DMA arrival order.
"""

import os
import sys

import numpy as np

for _p in ("/opt/trn_rl_repo",):
    if _p not in sys.path and os.path.isdir(_p):
        sys.path.insert(0, _p)

B = 8          # batches == cores
T = 4096       # tokens per batch
D = 1024       # model dim (contraction K)
N3 = 3072      # qkv output dim
P = 128        # SBUF partitions
NT = 512       # n-tile (one fp32 PSUM bank)
CH = 512       # token chunk (DMA granule for x)
KT = D // P        # 8 k-tiles
NN = N3 // NT      # 6 n-tiles
TT = CH // P       # 4 token sub-tiles per chunk
NCH = T // CH      # 8 chunks
SCALE = 8.0 / 8.0

_NC_CACHE = {}


def _build():
    import concourse.tile as tile
    from concourse import bacc, mybir

    bf16 = mybir.dt.bfloat16
    f32 = mybir.dt.float32

    nc = bacc.Bacc(
        "TRN2",
        target_bir_lowering=False,
        debug=False,
        enable_asserts=False,
        num_devices=B,
    )
    xt = nc.dram_tensor("xt", [D, T], bf16, kind="ExternalInput").ap()
    weff = nc.dram_tensor("weff", [D, N3], bf16, kind="ExternalInput").ap()
    biasr = nc.dram_tensor("biasr", [1, N3], bf16, kind="ExternalInput").ap()
    y = nc.dram_tensor("y", [T, N3], bf16, kind="ExternalOutput").ap()

    with tile.TileContext(nc) as tc:
        with tc.tile_pool(name="const", bufs=1) as const_pool, \
             tc.tile_pool(name="outp", bufs=6) as out_pool, \
             tc.tile_pool(name="ps", bufs=6, space="PSUM") as psum_pool, \
             tc.tile_pool(name="psb", bufs=2, space="PSUM") as psum_b_pool:

            # k-slice k of x lives at cols [k*T, (k+1)*T); of w at [k*N3, ...).
            x_sb = const_pool.tile([P, KT * T], bf16)
            w_sb = const_pool.tile([P, KT * N3], bf16)
            bias_sb = const_pool.tile([P, N3], f32)
            bias_row = const_pool.tile([1, N3], bf16)
            ones_sb = const_pool.tile([1, P], bf16)

            def load_x_c(c, eng):
                for k in range(KT):
                    eng.dma_start(
                        x_sb[:, k * T + c * CH : k * T + (c + 1) * CH],
                        xt[k * P : (k + 1) * P, c * CH : (c + 1) * CH],
                    )

            # Startup-critical: the head phase consumes (w[k,n0], x[k,c0])
            # in k order — w rides the sync ring, x rides the scalar ring so
            # the two streams land in parallel within ~1us of engine start.
            nc.scalar.dma_start(bias_row[:], biasr[:])
            for k in range(KT):
                nc.sync.dma_start(
                    w_sb[:, k * N3 : k * N3 + NT],
                    weff[k * P : (k + 1) * P, 0:NT],
                )
            load_x_c(0, nc.scalar)
            # After the head, compute consumes w k-major (k-outer, n-inner).
            # Stream the remaining w slices k-major, split across both rings
            # so the whole of w is resident by ~25us.
            for k in range(KT):
                eng = nc.scalar if k % 2 else nc.sync
                eng.dma_start(
                    w_sb[:, k * N3 + NT : (k + 1) * N3],
                    weff[k * P : (k + 1) * P, NT:N3],
                )
            # The rest of x queues behind the critical loads (HWDGE FIFO
            # order is the priority mechanism) as whole k-rows: 7KB per
            # partition line, which the DMA engines move at ~330GB/s vs
            # ~200GB/s for the 1KB-line chunk pieces.
            for k in range(KT):
                eng = nc.scalar if k % 2 else nc.sync
                eng.dma_start(
                    x_sb[:, k * T + CH : (k + 1) * T],
                    xt[k * P : (k + 1) * P, CH:T],
                )

            # One [128, 3072] output tile per token tile: the six bias-add
            # drains fill it slice by slice, then two half-row stores (3KB
            # partition lines) ship it on both rings in parallel.
            obs = {}

            def drain(ps, tg, n):
                if tg not in obs:
                    obs[tg] = out_pool.tile([P, N3], bf16, tag="ob", name="ob")
                ob = obs[tg]
                nc.vector.tensor_add(
                    ob[:, n * NT : (n + 1) * NT],
                    ps[:],
                    bias_sb[:, n * NT : (n + 1) * NT],
                )

            def store(tg, final=False):
                ob = obs.pop(tg)
                if final:
                    # Six slice-stores alternating rings: the first slices
                    # ship while the last drains still run, shrinking the
                    # kernel tail.
                    for n in range(NN):
                        eng = nc.sync if n % 2 else nc.scalar
                        eng.dma_start(
                            y[tg * P : (tg + 1) * P, n * NT : (n + 1) * NT],
                            ob[:, n * NT : (n + 1) * NT],
                        )
                    return
                half = N3 // 2
                nc.scalar.dma_start(
                    y[tg * P : (tg + 1) * P, 0:half], ob[:, 0:half]
                )
                nc.sync.dma_start(
                    y[tg * P : (tg + 1) * P, half:N3], ob[:, half:N3]
                )

            def mm(ps, tg, n, k):
                nc.tensor.matmul(
                    ps[:],
                    lhsT=x_sb[:, k * T + tg * P : k * T + (tg + 1) * P],
                    rhs=w_sb[:, k * N3 + n * NT : k * N3 + (n + 1) * NT],
                    start=(k == 0),
                    stop=(k == KT - 1),
                )

            # ~30 dependency-free warm-up matmuls on the ones tile keep the
            # PE busy from engine-up (~7us) until the first data lands
            # (~10us), so the HAM clock gate reaches 8/8 early.  Output goes
            # to a PSUM bank that is never read.
            nc.vector.memset(ones_sb[:], 1.0)
            wub = psum_b_pool.tile([P, P], f32, tag="psb", name="psb")
            for _ in range(30):
                nc.tensor.matmul(
                    wub[:], lhsT=ones_sb[:], rhs=ones_sb[:],
                    start=True, stop=True,
                )

            # Head phase (chunk 0, n=0): k-outer over 4 parallel PSUM groups
            # so the PE consumes tiles in exactly the DMA arrival order.
            pss = [psum_pool.tile([P, NT], f32, tag="ps", name="ps")
                   for _ in range(TT)]
            for k in range(KT):
                for t in range(TT):
                    mm(pss[t], t, 0, k)

            # Replicate bias across partitions on-chip: ones[1,128].T @
            # bias_row[1,512] fills [128,512].  12KB of HBM instead of 1.5MB,
            # and the PE does it in ~1.3us while the head phase wraps up.
            for n in range(NN):
                psb = psum_b_pool.tile([P, NT], f32, tag="psb", name="psb")
                nc.tensor.matmul(
                    psb[:],
                    lhsT=ones_sb[:],
                    rhs=bias_row[:, n * NT : (n + 1) * NT],
                    start=True,
                    stop=True,
                )
                nc.vector.tensor_copy(bias_sb[:, n * NT : (n + 1) * NT], psb[:])

            for t in range(TT):
                drain(pss[t], t, 0)

            # Everything else: k-outer / n-inner with one PSUM bank per n, so
            # the stationary operand (lhsT = x tile) is identical across the
            # n-consecutive matmuls — the PE's weight path stays quiet.
            def t_block(tg, n_lo):
                pss = [psum_pool.tile([P, NT], f32, tag="ps", name="ps")
                       for _ in range(NN - n_lo)]
                for k in range(KT):
                    for n in range(n_lo, NN):
                        mm(pss[n - n_lo], tg, n, k)
                for n in range(n_lo, NN):
                    drain(pss[n - n_lo], tg, n)
                store(tg, final=(tg == T // P - 1))

            for t in range(TT):
                t_block(t, 1)
            for c in range(1, NCH):
                for t in range(TT):
                    t_block(c * TT + t, 0)
    nc.compile()
    return nc


def _get_nc():
    if "v2" not in _NC_CACHE:
        _NC_CACHE["v2"] = _build()
    return _NC_CACHE["v2"]


def _prep_in_maps(inputs):
    import ml_dtypes

    bf16 = ml_dtypes.bfloat16

    x = np.asarray(inputs["x"], dtype=np.float32)
    weight = np.asarray(inputs["weight"], dtype=np.float32)
    bias = np.asarray(inputs["bias"], dtype=np.float32)
    aq = np.asarray(inputs["A_q_pool"], dtype=np.float32)
    bq = np.asarray(inputs["B_q_pool"], dtype=np.float32)
    av = np.asarray(inputs["A_v_pool"], dtype=np.float32)
    bv = np.asarray(inputs["B_v_pool"], dtype=np.float32)
    idx = np.asarray(inputs["idx"]).reshape(B, -1)[:, 0].astype(np.int64)

    wt64 = weight.T.astype(np.float64)  # [D, N3]
    biasr = np.ascontiguousarray(bias.reshape(1, N3))
    xts = x.transpose(0, 2, 1)  # [B, D, T] strided view

    in_maps = []
    for b in range(B):
        i = int(idx[b])
        weff = wt64.copy()
        weff[:, :D] += SCALE * (aq[i].astype(np.float64) @ bq[i].astype(np.float64))
        weff[:, N3 - D:] += SCALE * (av[i].astype(np.float64) @ bv[i].astype(np.float64))
        in_maps.append({
            "xt": np.ascontiguousarray(xts[b]).astype(bf16),
            "weff": weff.astype(np.float32).astype(bf16),
            "biasr": biasr.astype(bf16),
        })
    return in_maps


def _run(in_maps, trace=False, **kwargs):
    from concourse.bass_utils import run_bass_kernel_spmd

    nc = _get_nc()
    return run_bass_kernel_spmd(
        nc, in_maps, core_ids=list(range(B)), trace=trace, **kwargs
    )


def kernel(**inputs):
    res = _run(_prep_in_maps(inputs), trace=False)
    return np.stack(
        [np.asarray(r["y"], dtype=np.float32) for r in res.results], axis=0
    )
